# revision 14
# baseline (speedup 1.0000x reference)
"""Multi-head attention kernel for 8 TRN2 NeuronCores (Bass/Tile).

Problem: x[2,2048,1024], 16 heads x 64 dims, torch-style Linear weights.
Sharding: data parallel over batch (2) x tensor parallel over heads (16/4):
core c handles batch c//4, heads 4*(c%4) .. 4*(c%4)+3. Each core computes
its heads' attention output projected through its slice of wo, producing a
partial [2048, 1024] fp32 output; the host sums the 4 partials per batch
(the "all-reduce after wo").

Device dataflow per core (matmul operands bf16, fp32 accumulation):
  QT/KT = weight-slice projections in [d, t] layout (d on partitions)
  V     = projection in natural [s, d] layout, with a ones column appended
          per head so the P@V matmul also yields the softmax denominator
  S^T   = K^T.T @ Q^T per head ([s, t] layout, s on partitions)
  P^T   = exp(S^T / 8) via ScalarE (no max subtraction: logits are O(8))
  O^T   = V.T @ P^T accumulated over s in PSUM (row 64 = denominator)
  y     = (O^T / denom)^T @ wo-slice^T, partial over this core's heads

The attention stage is ScalarE(exp)-bound (~16.8M exp/core), so the s-loop
is software-pipelined in chunks of 2 s-tiles: the PE issues chunk c+1's
score matmuls before chunk c's PV matmuls so the in-order PE queue never
stalls on the exp dependency, and each exp is one [128,1024] PSUM-source
ACTIVATE (amortizes the per-instruction bubble). Final projection and the
softmax normalization for query-block tb are deferred into tb+1's pipeline.
"""

import sys

sys.path.insert(0, "/opt/trn_rl_repo")

from contextlib import ExitStack

import ml_dtypes
import numpy as np

import concourse.bass as bass
import concourse.tile as tile
from concourse import bacc, mybir
from concourse import bass_utils
from concourse.bass_interp import get_hw_module

BF16 = mybir.dt.bfloat16
F32 = mybir.dt.float32
EXP = mybir.ActivationFunctionType.Exp

N_EMBD = 1024
N_HEAD = 16
HEAD_DIM = 64

N_CORES = 8
HEADS_PER_CORE = 4
DH = HEADS_PER_CORE * HEAD_DIM  # 256
CH = 2  # s-tiles per exp chunk


def build_program(T=2048, C=N_EMBD, enable_asserts=False):
    nc = bacc.Bacc(
        "TRN2", target_bir_lowering=False, debug=False, enable_asserts=enable_asserts
    )

    xT = nc.dram_tensor("xT", [C, T], BF16, kind="ExternalInput").ap()
    wqT = nc.dram_tensor("wqT", [C, DH], BF16, kind="ExternalInput").ap()
    wkT = nc.dram_tensor("wkT", [C, DH], BF16, kind="ExternalInput").ap()
    wvT = nc.dram_tensor("wvT", [C, DH], BF16, kind="ExternalInput").ap()
    woT = nc.dram_tensor("woT", [DH, C], BF16, kind="ExternalInput").ap()
    y = nc.dram_tensor("y", [T, C], F32, kind="ExternalOutput").ap()

    n_ct = C // 128   # contraction tiles over embedding dim
    n_st = T // 128   # s tiles (key/value positions)
    n_tb = T // 512   # query blocks
    n_cb = C // 512   # output column blocks
    n_ck = n_st // CH  # exp chunks per (h, tb)

    scale = float(HEAD_DIM**-0.5)

    with tile.TileContext(nc) as tc, ExitStack() as ctx:
        statics = ctx.enter_context(tc.tile_pool(name="statics", bufs=1))
        pt_pool = ctx.enter_context(tc.tile_pool(name="pt", bufs=4))
        onorm_pool = ctx.enter_context(tc.tile_pool(name="onorm", bufs=6))
        small = ctx.enter_context(tc.tile_pool(name="small", bufs=6))
        out_stage = ctx.enter_context(tc.tile_pool(name="out_stage", bufs=4))

        psum_s = ctx.enter_context(tc.tile_pool(name="psum_s", bufs=2, space="PSUM"))
        psum_o = ctx.enter_context(tc.tile_pool(name="psum_o", bufs=2, space="PSUM"))
        psum_f = ctx.enter_context(tc.tile_pool(name="psum_f", bufs=2, space="PSUM"))

        # ---- static SBUF tensors ----
        xT_sb = statics.tile([128, n_ct, T], BF16)
        wq_sb = statics.tile([128, n_ct, DH], BF16)
        wk_sb = statics.tile([128, n_ct, DH], BF16)
        wv_sb = statics.tile([128, n_ct, DH], BF16)
        wo_sb = statics.tile([128, 2, C], BF16)
        qT_sb = statics.tile([128, 2, T], BF16)
        kT_sb = statics.tile([128, 2, T], BF16)
        v_sb = statics.tile([128, n_st, HEADS_PER_CORE, HEAD_DIM + 1], BF16)
        oT_sb = statics.tile([128, 2, T], BF16)
        ones_sb = statics.tile([1, 64], BF16)

        nc.sync.dma_start(out=wk_sb, in_=wkT.rearrange("(a p) d -> p a d", p=128))
        nc.sync.dma_start(out=wv_sb, in_=wvT.rearrange("(a p) d -> p a d", p=128))
        nc.sync.dma_start(out=wq_sb, in_=wqT.rearrange("(a p) d -> p a d", p=128))
        nc.sync.dma_start(out=wo_sb, in_=woT.rearrange("(a p) c -> p a c", p=128))
        xT_chunked = xT.rearrange("(a p) t -> p a t", p=128)
        for ct in range(n_ct):
            nc.sync.dma_start(out=xT_sb[:, ct, :], in_=xT_chunked[:, ct, :])
        nc.vector.memset(ones_sb, 1.0)
        nc.vector.memset(v_sb[:, :, :, HEAD_DIM : HEAD_DIM + 1], 1.0)

        # ---- projections (K, V first; attention waits on all of them) ----
        qk_projs = ((wk_sb, kT_sb), (wq_sb, qT_sb))
        proj_scope = nc.named_scope("proj")
        proj_scope.__enter__()
        for w_sb, dst in qk_projs[:1]:
            for chunk in range(2):
                for tb in range(n_tb):
                    ps = psum_f.tile([128, 512], F32, tag="f")
                    for ct in range(n_ct):
                        nc.tensor.matmul(
                            ps,
                            lhsT=w_sb[:, ct, chunk * 128 : (chunk + 1) * 128],
                            rhs=xT_sb[:, ct, tb * 512 : (tb + 1) * 512],
                            start=(ct == 0),
                            stop=(ct == n_ct - 1),
                        )
                    nc.vector.tensor_copy(dst[:, chunk, tb * 512 : (tb + 1) * 512], ps)

        for st in range(n_st):
            ps = psum_f.tile([128, 512], F32, tag="f")
            for ct in range(n_ct):
                nc.tensor.matmul(
                    ps[:, 0:DH],
                    lhsT=xT_sb[:, ct, st * 128 : (st + 1) * 128],
                    rhs=wv_sb[:, ct, :],
                    start=(ct == 0),
                    stop=(ct == n_ct - 1),
                )
            nc.vector.tensor_copy(
                v_sb[:, st, :, 0:HEAD_DIM],
                ps[:, 0:DH].rearrange("p (h d) -> p h d", h=HEADS_PER_CORE),
            )

        for w_sb, dst in qk_projs[1:]:
            for chunk in range(2):
                for tb in range(n_tb):
                    ps = psum_f.tile([128, 512], F32, tag="f")
                    for ct in range(n_ct):
                        nc.tensor.matmul(
                            ps,
                            lhsT=w_sb[:, ct, chunk * 128 : (chunk + 1) * 128],
                            rhs=xT_sb[:, ct, tb * 512 : (tb + 1) * 512],
                            start=(ct == 0),
                            stop=(ct == n_ct - 1),
                        )
                    nc.vector.tensor_copy(dst[:, chunk, tb * 512 : (tb + 1) * 512], ps)

        proj_scope.__exit__(None, None, None)
        # ---- attention: software-pipelined chunk loop ----
        # chunk records: (h, tb, c, ps, pt, o_ps); deferred: list of
        # (slot_index, emit_fn) executed at the start of pipeline slot i.
        heads = list(range(HEADS_PER_CORE))
        chunk_list = []
        for tb in range(n_tb):
            for h in heads:
                for c in range(n_ck):
                    chunk_list.append((tb, h, c))
        n_total = len(chunk_list)

        deferred = {}

        def defer(slot, fn):
            deferred.setdefault(slot, []).append(fn)

        o_ps_map = {}
        denom_map = {}

        def emit_S_ACT(i):
            tb, h, c = chunk_list[i]
            chunk_hd, dlo = h // 2, (h % 2) * 64
            if c == 0:
                o_ps = psum_o.tile([HEAD_DIM + 1, 512], F32, tag="o")
                o_ps_map[(tb, h)] = o_ps
            with nc.named_scope("S"):
                ps = psum_s.tile([128, CH * 512], F32, tag="s")
                for j in range(CH):
                    st = c * CH + j
                    nc.tensor.matmul(
                        ps[:, j * 512 : (j + 1) * 512],
                        lhsT=kT_sb[dlo : dlo + 64, chunk_hd, st * 128 : (st + 1) * 128],
                        rhs=qT_sb[dlo : dlo + 64, chunk_hd, tb * 512 : (tb + 1) * 512],
                        start=True,
                        stop=True,
                    )
            with nc.named_scope("exp"):
                pt = pt_pool.tile([128, CH, 512], BF16)
                nc.scalar.activation(
                    pt.rearrange("p a b -> p (a b)"), ps, EXP, scale=scale
                )
            return ps, pt

        def emit_O(i, pt):
            tb, h, c = chunk_list[i]
            o_ps = o_ps_map[(tb, h)]
            with nc.named_scope("O"):
                for j in range(CH):
                    st = c * CH + j
                    nc.tensor.matmul(
                        o_ps,
                        lhsT=v_sb[:, st, h, :],
                        rhs=pt[:, j, :],
                        start=(c == 0 and j == 0),
                        stop=(c == n_ck - 1 and j == CH - 1),
                    )
            if c == n_ck - 1:
                # head (tb, h) fully accumulated: stash unnormalized O
                o_unnorm = onorm_pool.tile([64, 512], BF16, tag="ou")
                nc.vector.tensor_copy(o_unnorm, o_ps[0:64, :])
                o_unnorm_map[(tb, h)] = o_unnorm

        o_unnorm_map = {}

        def emit_norm(tb, h):
            ns = nc.named_scope("norm"); ns.__enter__()
            chunk_hd, dlo = h // 2, (h % 2) * 64
            o_ps = o_ps_map[(tb, h)]
            denom_f = small.tile([1, 512], F32, tag="denom_f")
            nc.vector.tensor_copy(denom_f, o_ps[64:65, :])
            recip_f = small.tile([1, 512], F32, tag="recip_f")
            nc.vector.reciprocal_approx_fast(recip_f, denom_f)
            recip = small.tile([1, 512], BF16, tag="recip")
            nc.vector.tensor_copy(recip, recip_f)
            rep = psum_f.tile([128, 512], F32, tag="f")
            nc.tensor.matmul(
                rep[0:64, :], lhsT=ones_sb, rhs=recip, start=True, stop=True
            )
            rep_sb = small.tile([64, 512], BF16, tag="rep")
            nc.vector.tensor_copy(rep_sb, rep[0:64, :])
            nc.vector.tensor_mul(
                oT_sb[dlo : dlo + 64, chunk_hd, tb * 512 : (tb + 1) * 512],
                o_unnorm_map[(tb, h)],
                rep_sb,
            )
            ns.__exit__(None, None, None)

        def emit_final(tb):
            ns = nc.named_scope("final"); ns.__enter__()
            # y partial for query block tb: 4 t-tiles of 128
            y_tiled = y.rearrange("(tt p) c -> p tt c", p=128)
            for tt in range(tb * 4, tb * 4 + 4):
                for cb in range(n_cb):
                    ps = psum_f.tile([128, 512], F32, tag="f")
                    for chunk in range(2):
                        nc.tensor.matmul(
                            ps,
                            lhsT=oT_sb[:, chunk, tt * 128 : (tt + 1) * 128],
                            rhs=wo_sb[:, chunk, cb * 512 : (cb + 1) * 512],
                            start=(chunk == 0),
                            stop=(chunk == 1),
                        )
                    st_out = out_stage.tile([128, 512], F32)
                    nc.vector.tensor_copy(st_out, ps)
                    nc.sync.dma_start(
                        out=y_tiled[:, tt, cb * 512 : (cb + 1) * 512], in_=st_out
                    )
            ns.__exit__(None, None, None)

        # schedule: at slot i emit S+ACT(i) then O(i-1); normalization for tb
        # lands ~3 chunks into tb+1, final projection ~6 chunks in.
        per_tb = HEADS_PER_CORE * n_ck
        for tb in range(n_tb):
            for h in heads:
                head_end = (tb * HEADS_PER_CORE + h + 1) * n_ck
                last = n_total + 1  # after the final pending O pops
                defer(min(head_end + 3, last), lambda tb=tb, h=h: emit_norm(tb, h))
            end_slot = (tb + 1) * per_tb
            defer(min(end_slot + 6, last), lambda tb=tb: emit_final(tb))

        from collections import deque

        pending = deque()
        DEPTH = 2
        for i in range(n_total + DEPTH):
            if i < n_total:
                ps, pt = emit_S_ACT(i)
                pending.append((i, pt))
            if len(pending) > DEPTH or i >= n_total:
                j, jpt = pending.popleft()
                emit_O(j, jpt)
            for fn in deferred.get(i, ()):
                fn()

    nc.compile()
    return nc


def make_core_inputs(x, wq, wk, wv, wo):
    """Shard + pre-layout the full inputs into 8 per-core input maps."""
    bf = ml_dtypes.bfloat16
    in_maps = []
    for core in range(N_CORES):
        b = core // 4
        g = core % 4
        lo, hi = g * DH, (g + 1) * DH
        in_maps.append(
            {
                "xT": np.ascontiguousarray(x[b].T).astype(bf),
                "wqT": np.ascontiguousarray(wq[lo:hi, :].T).astype(bf),
                "wkT": np.ascontiguousarray(wk[lo:hi, :].T).astype(bf),
                "wvT": np.ascontiguousarray(wv[lo:hi, :].T).astype(bf),
                "woT": np.ascontiguousarray(wo[:, lo:hi].T).astype(bf),
            }
        )
    return in_maps


_PROGRAM_CACHE = {}


def _get_program():
    if "nc" not in _PROGRAM_CACHE:
        nc = build_program()
        nc.m = get_hw_module(nc.m)
        _PROGRAM_CACHE["nc"] = nc
    return _PROGRAM_CACHE["nc"]


def run_sharded(in_maps, trace=False):
    nc = _get_program()
    return bass_utils.run_bass_kernel_spmd(
        nc, in_maps, core_ids=list(range(N_CORES)), trace=trace
    )


def kernel(x, wq, wk, wv, wo):
    x = np.asarray(x, dtype=np.float32)
    wq = np.asarray(wq, dtype=np.float32)
    wk = np.asarray(wk, dtype=np.float32)
    wv = np.asarray(wv, dtype=np.float32)
    wo = np.asarray(wo, dtype=np.float32)

    in_maps = make_core_inputs(x, wq, wk, wv, wo)
    res = run_sharded(in_maps)

    B, T, C = x.shape
    out = np.zeros((B, T, C), dtype=np.float32)
    for core in range(N_CORES):
        out[core // 4] += res.results[core]["y"]
    return out


if __name__ == "__main__":
    rng = np.random.default_rng(0)
    x = rng.standard_normal((2, 2048, 1024), dtype=np.float32)
    s = 1.0 / np.sqrt(N_EMBD)
    ws = [rng.standard_normal((1024, 1024), dtype=np.float32) * s for _ in range(4)]
    out = kernel(x, *ws)
    print("out", out.shape, out.dtype, float(np.abs(out).max()))


# revision 17
# speedup vs baseline: 1.0326x; 1.0326x over previous
"""Multi-head attention kernel for 8 TRN2 NeuronCores (Bass/Tile).

Problem: x[2,2048,1024], 16 heads x 64 dims, torch-style Linear weights.
Sharding: data parallel over batch (2) x tensor parallel over heads (16/4):
core c handles batch c//4, heads 4*(c%4) .. 4*(c%4)+3. Each core computes
its heads' attention output projected through its slice of wo, producing a
partial [2048, 1024] fp32 output; the host sums the 4 partials per batch
(the "all-reduce after wo").

Device dataflow per core (matmul operands bf16, fp32 accumulation):
  QT/KT = weight-slice projections in [d, t] layout (d on partitions)
  V     = projection in natural [s, d] layout, with a ones column appended
          per head so the P@V matmul also yields the softmax denominator
  S^T   = K^T.T @ Q^T per head ([s, t] layout, s on partitions)
  P^T   = exp(S^T / 8) via ScalarE (no max subtraction: logits are O(8))
  O^T   = V.T @ P^T accumulated over s in PSUM (row 64 = denominator)
  y     = (O^T / denom)^T @ wo-slice^T, partial over this core's heads

The attention stage is ScalarE(exp)-bound (~16.8M exp/core), so the s-loop
is software-pipelined in chunks of 2 s-tiles: the PE issues chunk c+1's
score matmuls before chunk c's PV matmuls so the in-order PE queue never
stalls on the exp dependency, and each exp is one [128,1024] PSUM-source
ACTIVATE (amortizes the per-instruction bubble). Final projection and the
softmax normalization for query-block tb are deferred into tb+1's pipeline.
"""

import sys

sys.path.insert(0, "/opt/trn_rl_repo")

from contextlib import ExitStack

import ml_dtypes
import numpy as np

import concourse.bass as bass
import concourse.tile as tile
from concourse import bacc, mybir
from concourse import bass_utils
from concourse.bass_interp import get_hw_module

BF16 = mybir.dt.bfloat16
F32 = mybir.dt.float32
EXP = mybir.ActivationFunctionType.Exp

N_EMBD = 1024
N_HEAD = 16
HEAD_DIM = 64

N_CORES = 8
HEADS_PER_CORE = 4
DH = HEADS_PER_CORE * HEAD_DIM  # 256
CH = 2  # s-tiles per exp chunk


def build_program(T=2048, C=N_EMBD, enable_asserts=False):
    nc = bacc.Bacc(
        "TRN2", target_bir_lowering=False, debug=False, enable_asserts=enable_asserts
    )

    xT = nc.dram_tensor("xT", [C, T], BF16, kind="ExternalInput").ap()
    wqT = nc.dram_tensor("wqT", [C, DH], BF16, kind="ExternalInput").ap()
    wkT = nc.dram_tensor("wkT", [C, DH], BF16, kind="ExternalInput").ap()
    wvT = nc.dram_tensor("wvT", [C, DH], BF16, kind="ExternalInput").ap()
    woT = nc.dram_tensor("woT", [DH, C], BF16, kind="ExternalInput").ap()
    y = nc.dram_tensor("y", [T, C], F32, kind="ExternalOutput").ap()

    n_ct = C // 128   # contraction tiles over embedding dim
    n_st = T // 128   # s tiles (key/value positions)
    n_tb = T // 512   # query blocks
    n_cb = C // 512   # output column blocks
    n_ck = n_st // CH  # exp chunks per (h, tb)

    scale = float(HEAD_DIM**-0.5)

    with tile.TileContext(nc) as tc, ExitStack() as ctx:
        statics = ctx.enter_context(tc.tile_pool(name="statics", bufs=1))
        pt_pool = ctx.enter_context(tc.tile_pool(name="pt", bufs=4))
        onorm_pool = ctx.enter_context(tc.tile_pool(name="onorm", bufs=6))
        small = ctx.enter_context(tc.tile_pool(name="small", bufs=6))
        out_stage = ctx.enter_context(tc.tile_pool(name="out_stage", bufs=4))

        psum_s = ctx.enter_context(tc.tile_pool(name="psum_s", bufs=2, space="PSUM"))
        psum_o = ctx.enter_context(tc.tile_pool(name="psum_o", bufs=2, space="PSUM"))
        psum_f = ctx.enter_context(tc.tile_pool(name="psum_f", bufs=2, space="PSUM"))

        # ---- static SBUF tensors ----
        xT_sb = statics.tile([128, n_ct, T], BF16)
        wq_sb = statics.tile([128, n_ct, DH], BF16)
        wk_sb = statics.tile([128, n_ct, DH], BF16)
        wv_sb = statics.tile([128, n_ct, DH], BF16)
        wo_sb = statics.tile([128, 2, C], BF16)
        qT_sb = statics.tile([128, 2, T], BF16)
        kT_sb = statics.tile([128, 2, T], BF16)
        v_sb = statics.tile([128, n_st, HEADS_PER_CORE, HEAD_DIM + 1], BF16)
        oT_sb = statics.tile([128, 2, T], BF16)
        ones_sb = statics.tile([1, 64], BF16)

        nc.sync.dma_start(out=wk_sb, in_=wkT.rearrange("(a p) d -> p a d", p=128))
        nc.sync.dma_start(out=wv_sb, in_=wvT.rearrange("(a p) d -> p a d", p=128))
        xT_chunked = xT.rearrange("(a p) t -> p a t", p=128)
        for ct in range(n_ct):
            half = T // 2
            nc.sync.dma_start(out=xT_sb[:, ct, 0:half], in_=xT_chunked[:, ct, 0:half])
            nc.sync.dma_start(out=xT_sb[:, ct, half:T], in_=xT_chunked[:, ct, half:T])
        nc.sync.dma_start(out=wq_sb, in_=wqT.rearrange("(a p) d -> p a d", p=128))
        nc.sync.dma_start(out=wo_sb, in_=woT.rearrange("(a p) c -> p a c", p=128))
        nc.vector.memset(ones_sb, 1.0)
        nc.vector.memset(v_sb[:, :, :, HEAD_DIM : HEAD_DIM + 1], 1.0)

        # ---- projections (overlapped with the xT DMA fill) ----
        proj_scope = nc.named_scope("proj")
        proj_scope.__enter__()

        def emit_qk_group(w_sb, dst, chunk, tb):
            ps = psum_f.tile([128, 512], F32, tag="f")
            for ct in range(n_ct):
                nc.tensor.matmul(
                    ps,
                    lhsT=w_sb[:, ct, chunk * 128 : (chunk + 1) * 128],
                    rhs=xT_sb[:, ct, tb * 512 : (tb + 1) * 512],
                    start=(ct == 0),
                    stop=(ct == n_ct - 1),
                )
            nc.vector.tensor_copy(dst[:, chunk, tb * 512 : (tb + 1) * 512], ps)

        # K chunk0 with ct-outer accumulation into 4 parallel query-block
        # groups, so the matmuls chase the per-chunk xT DMAs as they land
        kps0 = psum_s.tile([128, 1024], F32, tag="s")
        kps1 = psum_s.tile([128, 1024], F32, tag="s")
        kps = [kps0, kps1]
        for ct in range(n_ct):
            for tb in range(n_tb):
                nc.tensor.matmul(
                    kps[tb // 2][:, (tb % 2) * 512 : (tb % 2 + 1) * 512],
                    lhsT=wk_sb[:, ct, 0:128],
                    rhs=xT_sb[:, ct, tb * 512 : (tb + 1) * 512],
                    start=(ct == 0),
                    stop=(ct == n_ct - 1),
                )
        for tb in range(n_tb):
            nc.vector.tensor_copy(
                kT_sb[:, 0, tb * 512 : (tb + 1) * 512],
                kps[tb // 2][:, (tb % 2) * 512 : (tb % 2 + 1) * 512],
            )

        # V projection (natural layout + ones column)
        for st in range(n_st):
            ps = psum_f.tile([128, 512], F32, tag="f")
            for ct in range(n_ct):
                nc.tensor.matmul(
                    ps[:, 0:DH],
                    lhsT=xT_sb[:, ct, st * 128 : (st + 1) * 128],
                    rhs=wv_sb[:, ct, :],
                    start=(ct == 0),
                    stop=(ct == n_ct - 1),
                )
            nc.vector.tensor_copy(
                v_sb[:, st, :, 0:HEAD_DIM],
                ps[:, 0:DH].rearrange("p (h d) -> p h d", h=HEADS_PER_CORE),
            )

        # first query block for chunk0 heads; the rest are deferred into the
        # early attention pipeline slots
        emit_qk_group(wq_sb, qT_sb, 0, 0)
        proj_scope.__exit__(None, None, None)
        # ---- attention: software-pipelined chunk loop ----
        # chunk records: (h, tb, c, ps, pt, o_ps); deferred: list of
        # (slot_index, emit_fn) executed at the start of pipeline slot i.
        heads = list(range(HEADS_PER_CORE))
        chunk_list = []
        for tb in range(n_tb):
            for h in heads:
                for c in range(n_ck):
                    chunk_list.append((tb, h, c))
        n_total = len(chunk_list)

        deferred = {}

        def defer(slot, fn):
            deferred.setdefault(slot, []).append(fn)

        o_ps_map = {}
        denom_map = {}

        def emit_S_ACT(i):
            tb, h, c = chunk_list[i]
            chunk_hd, dlo = h // 2, (h % 2) * 64
            if c == 0:
                o_ps = psum_o.tile([HEAD_DIM + 1, 512], F32, tag="o")
                o_ps_map[(tb, h)] = o_ps
            with nc.named_scope("S"):
                ps = psum_s.tile([128, CH * 512], F32, tag="s")
                for j in range(CH):
                    st = c * CH + j
                    nc.tensor.matmul(
                        ps[:, j * 512 : (j + 1) * 512],
                        lhsT=kT_sb[dlo : dlo + 64, chunk_hd, st * 128 : (st + 1) * 128],
                        rhs=qT_sb[dlo : dlo + 64, chunk_hd, tb * 512 : (tb + 1) * 512],
                        start=True,
                        stop=True,
                    )
            with nc.named_scope("exp"):
                pt = pt_pool.tile([128, CH, 512], BF16)
                nc.scalar.activation(
                    pt.rearrange("p a b -> p (a b)"), ps, EXP, scale=scale
                )
            return ps, pt

        def emit_O(i, pt):
            tb, h, c = chunk_list[i]
            o_ps = o_ps_map[(tb, h)]
            with nc.named_scope("O"):
                for j in range(CH):
                    st = c * CH + j
                    nc.tensor.matmul(
                        o_ps,
                        lhsT=v_sb[:, st, h, :],
                        rhs=pt[:, j, :],
                        start=(c == 0 and j == 0),
                        stop=(c == n_ck - 1 and j == CH - 1),
                    )
            if c == n_ck - 1:
                # head (tb, h) fully accumulated: stash unnormalized O
                o_unnorm = onorm_pool.tile([64, 512], BF16, tag="ou")
                nc.vector.tensor_copy(o_unnorm, o_ps[0:64, :])
                o_unnorm_map[(tb, h)] = o_unnorm

        o_unnorm_map = {}

        def emit_norm(tb, h):
            ns = nc.named_scope("norm"); ns.__enter__()
            chunk_hd, dlo = h // 2, (h % 2) * 64
            o_ps = o_ps_map[(tb, h)]
            denom_f = small.tile([1, 512], F32, tag="denom_f")
            nc.vector.tensor_copy(denom_f, o_ps[64:65, :])
            recip_f = small.tile([1, 512], F32, tag="recip_f")
            nc.vector.reciprocal_approx_fast(recip_f, denom_f)
            recip = small.tile([1, 512], BF16, tag="recip")
            nc.vector.tensor_copy(recip, recip_f)
            rep = psum_f.tile([128, 512], F32, tag="f")
            nc.tensor.matmul(
                rep[0:64, :], lhsT=ones_sb, rhs=recip, start=True, stop=True
            )
            rep_sb = small.tile([64, 512], BF16, tag="rep")
            nc.vector.tensor_copy(rep_sb, rep[0:64, :])
            nc.vector.tensor_mul(
                oT_sb[dlo : dlo + 64, chunk_hd, tb * 512 : (tb + 1) * 512],
                o_unnorm_map[(tb, h)],
                rep_sb,
            )
            ns.__exit__(None, None, None)

        def emit_final(tb):
            ns = nc.named_scope("final"); ns.__enter__()
            # y partial for query block tb: 4 t-tiles of 128
            y_tiled = y.rearrange("(tt p) c -> p tt c", p=128)
            for tt in range(tb * 4, tb * 4 + 4):
                for cb in range(n_cb):
                    ps = psum_f.tile([128, 512], F32, tag="f")
                    for chunk in range(2):
                        nc.tensor.matmul(
                            ps,
                            lhsT=oT_sb[:, chunk, tt * 128 : (tt + 1) * 128],
                            rhs=wo_sb[:, chunk, cb * 512 : (cb + 1) * 512],
                            start=(chunk == 0),
                            stop=(chunk == 1),
                        )
                    st_out = out_stage.tile([128, 512], F32)
                    nc.vector.tensor_copy(st_out, ps)
                    nc.sync.dma_start(
                        out=y_tiled[:, tt, cb * 512 : (cb + 1) * 512], in_=st_out
                    )
            ns.__exit__(None, None, None)

        # schedule: at slot i emit S+ACT(i) then O(i-1); normalization for tb
        # lands ~3 chunks into tb+1, final projection ~6 chunks in.
        # remaining projection groups dripped into early pipeline slots
        rest = [(wk_sb, kT_sb, 1, tb) for tb in range(n_tb)]
        rest += [(wq_sb, qT_sb, 1, 0)]
        for tb in range(1, n_tb):
            rest += [(wq_sb, qT_sb, 0, tb), (wq_sb, qT_sb, 1, tb)]
        for idx, (w_sb_, dst_, chunk_, tb_) in enumerate(rest):
            defer(
                1 + 2 * idx,
                lambda w=w_sb_, d=dst_, c=chunk_, t=tb_: emit_qk_group(w, d, c, t),
            )

        per_tb = HEADS_PER_CORE * n_ck
        for tb in range(n_tb):
            for h in heads:
                head_end = (tb * HEADS_PER_CORE + h + 1) * n_ck
                last = n_total + 1  # after the final pending O pops
                defer(min(head_end + 3, last), lambda tb=tb, h=h: emit_norm(tb, h))
            end_slot = (tb + 1) * per_tb
            defer(min(end_slot + 6, last), lambda tb=tb: emit_final(tb))

        from collections import deque

        pending = deque()
        DEPTH = 2
        for i in range(n_total + DEPTH):
            if i < n_total:
                ps, pt = emit_S_ACT(i)
                pending.append((i, pt))
            if len(pending) > DEPTH or i >= n_total:
                j, jpt = pending.popleft()
                emit_O(j, jpt)
            for fn in deferred.get(i, ()):
                fn()

    nc.compile()
    return nc


def make_core_inputs(x, wq, wk, wv, wo):
    """Shard + pre-layout the full inputs into 8 per-core input maps."""
    bf = ml_dtypes.bfloat16
    in_maps = []
    for core in range(N_CORES):
        b = core // 4
        g = core % 4
        lo, hi = g * DH, (g + 1) * DH
        in_maps.append(
            {
                "xT": np.ascontiguousarray(x[b].T).astype(bf),
                "wqT": np.ascontiguousarray(wq[lo:hi, :].T).astype(bf),
                "wkT": np.ascontiguousarray(wk[lo:hi, :].T).astype(bf),
                "wvT": np.ascontiguousarray(wv[lo:hi, :].T).astype(bf),
                "woT": np.ascontiguousarray(wo[:, lo:hi].T).astype(bf),
            }
        )
    return in_maps


_PROGRAM_CACHE = {}


def _get_program():
    if "nc" not in _PROGRAM_CACHE:
        nc = build_program()
        nc.m = get_hw_module(nc.m)
        _PROGRAM_CACHE["nc"] = nc
    return _PROGRAM_CACHE["nc"]


def run_sharded(in_maps, trace=False):
    nc = _get_program()
    return bass_utils.run_bass_kernel_spmd(
        nc, in_maps, core_ids=list(range(N_CORES)), trace=trace
    )


def kernel(x, wq, wk, wv, wo):
    x = np.asarray(x, dtype=np.float32)
    wq = np.asarray(wq, dtype=np.float32)
    wk = np.asarray(wk, dtype=np.float32)
    wv = np.asarray(wv, dtype=np.float32)
    wo = np.asarray(wo, dtype=np.float32)

    in_maps = make_core_inputs(x, wq, wk, wv, wo)
    res = run_sharded(in_maps)

    B, T, C = x.shape
    out = np.zeros((B, T, C), dtype=np.float32)
    for core in range(N_CORES):
        out[core // 4] += res.results[core]["y"]
    return out


if __name__ == "__main__":
    rng = np.random.default_rng(0)
    x = rng.standard_normal((2, 2048, 1024), dtype=np.float32)
    s = 1.0 / np.sqrt(N_EMBD)
    ws = [rng.standard_normal((1024, 1024), dtype=np.float32) * s for _ in range(4)]
    out = kernel(x, *ws)
    print("out", out.shape, out.dtype, float(np.abs(out).max()))


# revision 21
# speedup vs baseline: 1.1036x; 1.0688x over previous
"""Multi-head attention kernel for 8 TRN2 NeuronCores (Bass/Tile).

Problem: x[2,2048,1024], 16 heads x 64 dims, torch-style Linear weights.
Sharding: data parallel over batch (2) x tensor parallel over heads (16/4):
core c handles batch c//4, heads 4*(c%4) .. 4*(c%4)+3. Each core computes
its heads' attention output projected through its slice of wo, producing a
partial [2048, 1024] fp32 output; the host sums the 4 partials per batch
(the "all-reduce after wo").

Device dataflow per core (matmul operands bf16, fp32 accumulation):
  QT/KT = weight-slice projections in [d, t] layout (d on partitions)
  V     = projection in natural [s, d] layout, with a ones column appended
          per head so the P@V matmul also yields the softmax denominator
  S^T   = K^T.T @ Q^T per head ([s, t] layout, s on partitions)
  P^T   = exp(S^T / 8) via ScalarE (no max subtraction: logits are O(8))
  O^T   = V.T @ P^T accumulated over s in PSUM (row 64 = denominator)
  y     = (O^T / denom)^T @ wo-slice^T, partial over this core's heads

The attention stage is ScalarE(exp)-bound (~16.8M exp/core), so the s-loop
is software-pipelined in chunks of 2 s-tiles: the PE issues chunk c+1's
score matmuls before chunk c's PV matmuls so the in-order PE queue never
stalls on the exp dependency, and each exp is one [128,1024] PSUM-source
ACTIVATE (amortizes the per-instruction bubble). Final projection and the
softmax normalization for query-block tb are deferred into tb+1's pipeline.
"""

import sys

sys.path.insert(0, "/opt/trn_rl_repo")

from contextlib import ExitStack

import ml_dtypes
import numpy as np

import concourse.bass as bass
import concourse.tile as tile
from concourse import bacc, mybir
from concourse import bass_utils
from concourse.bass_interp import get_hw_module

BF16 = mybir.dt.bfloat16
F32 = mybir.dt.float32
EXP = mybir.ActivationFunctionType.Exp

N_EMBD = 1024
N_HEAD = 16
HEAD_DIM = 64

N_CORES = 8
HEADS_PER_CORE = 4
DH = HEADS_PER_CORE * HEAD_DIM  # 256
CH = 2  # s-tiles per exp chunk


def build_program(T=2048, C=N_EMBD, enable_asserts=False):
    nc = bacc.Bacc(
        "TRN2", target_bir_lowering=False, debug=False, enable_asserts=enable_asserts
    )

    xT = nc.dram_tensor("xT", [C, T], BF16, kind="ExternalInput").ap()
    wqT = nc.dram_tensor("wqT", [C, DH], BF16, kind="ExternalInput").ap()
    wkT = nc.dram_tensor("wkT", [C, DH], BF16, kind="ExternalInput").ap()
    wvT = nc.dram_tensor("wvT", [C, DH], BF16, kind="ExternalInput").ap()
    woT = nc.dram_tensor("woT", [DH, C], BF16, kind="ExternalInput").ap()
    y = nc.dram_tensor("y", [T, C], F32, kind="ExternalOutput").ap()

    n_ct = C // 128   # contraction tiles over embedding dim
    n_st = T // 128   # s tiles (key/value positions)
    n_tb = T // 512   # query blocks
    n_cb = C // 512   # output column blocks
    n_ck = n_st // CH  # exp chunks per (h, tb)

    scale = float(HEAD_DIM**-0.5)

    with tile.TileContext(nc) as tc, ExitStack() as ctx:
        statics = ctx.enter_context(tc.tile_pool(name="statics", bufs=1))
        pt_pool = ctx.enter_context(tc.tile_pool(name="pt", bufs=4))
        onorm_pool = ctx.enter_context(tc.tile_pool(name="onorm", bufs=6))
        small = ctx.enter_context(tc.tile_pool(name="small", bufs=6))
        out_stage = ctx.enter_context(tc.tile_pool(name="out_stage", bufs=4))

        psum_s = ctx.enter_context(tc.tile_pool(name="psum_s", bufs=2, space="PSUM"))
        psum_o = ctx.enter_context(tc.tile_pool(name="psum_o", bufs=2, space="PSUM"))
        psum_f = ctx.enter_context(tc.tile_pool(name="psum_f", bufs=2, space="PSUM"))

        # ---- static SBUF tensors ----
        xT_sb = statics.tile([128, n_ct, T], BF16)
        wq_sb = statics.tile([128, n_ct, DH], BF16)
        wk_sb = statics.tile([128, n_ct, DH], BF16)
        wv_sb = statics.tile([128, n_ct, DH], BF16)
        wo_sb = statics.tile([128, 2, C], BF16)
        qT_sb = statics.tile([128, 2, T], BF16)
        kT_sb = statics.tile([128, 2, T], BF16)
        v_sb = statics.tile([128, n_st, HEADS_PER_CORE, HEAD_DIM + 1], BF16)
        oT_sb = statics.tile([128, 2, T], BF16)
        ones_sb = statics.tile([1, 64], BF16)

        nc.sync.dma_start(out=wk_sb, in_=wkT.rearrange("(a p) d -> p a d", p=128))
        nc.sync.dma_start(out=wv_sb, in_=wvT.rearrange("(a p) d -> p a d", p=128))
        xT_chunked = xT.rearrange("(a p) t -> p a t", p=128)
        for ct in range(n_ct):
            half = T // 2
            nc.sync.dma_start(out=xT_sb[:, ct, 0:half], in_=xT_chunked[:, ct, 0:half])
            nc.sync.dma_start(out=xT_sb[:, ct, half:T], in_=xT_chunked[:, ct, half:T])
        nc.sync.dma_start(out=wq_sb, in_=wqT.rearrange("(a p) d -> p a d", p=128))
        nc.sync.dma_start(out=wo_sb, in_=woT.rearrange("(a p) c -> p a c", p=128))
        nc.vector.memset(ones_sb, 1.0)
        nc.vector.memset(v_sb[:, :, :, HEAD_DIM : HEAD_DIM + 1], 1.0)

        # ---- projections (overlapped with the xT DMA fill) ----
        proj_scope = nc.named_scope("proj")
        proj_scope.__enter__()

        def emit_qk_group(w_sb, dst, chunk, tb):
            ps = psum_f.tile([128, 512], F32, tag="f")
            for ct in range(n_ct):
                nc.tensor.matmul(
                    ps,
                    lhsT=w_sb[:, ct, chunk * 128 : (chunk + 1) * 128],
                    rhs=xT_sb[:, ct, tb * 512 : (tb + 1) * 512],
                    start=(ct == 0),
                    stop=(ct == n_ct - 1),
                )
            nc.vector.tensor_copy(dst[:, chunk, tb * 512 : (tb + 1) * 512], ps)

        # K chunk0 with ct-outer accumulation into 4 parallel query-block
        # groups, so the matmuls chase the per-chunk xT DMAs as they land
        kps0 = psum_s.tile([128, 1024], F32, tag="s")
        kps1 = psum_s.tile([128, 1024], F32, tag="s")
        kps = [kps0, kps1]
        for ct in range(n_ct):
            for tb in range(n_tb):
                nc.tensor.matmul(
                    kps[tb // 2][:, (tb % 2) * 512 : (tb % 2 + 1) * 512],
                    lhsT=wk_sb[:, ct, 0:128],
                    rhs=xT_sb[:, ct, tb * 512 : (tb + 1) * 512],
                    start=(ct == 0),
                    stop=(ct == n_ct - 1),
                )
        for tb in range(n_tb):
            nc.vector.tensor_copy(
                kT_sb[:, 0, tb * 512 : (tb + 1) * 512],
                kps[tb // 2][:, (tb % 2) * 512 : (tb % 2 + 1) * 512],
            )

        # V projection (natural layout + ones column)
        for st in range(n_st):
            ps = psum_f.tile([128, 512], F32, tag="f")
            for ct in range(n_ct):
                nc.tensor.matmul(
                    ps[:, 0:DH],
                    lhsT=xT_sb[:, ct, st * 128 : (st + 1) * 128],
                    rhs=wv_sb[:, ct, :],
                    start=(ct == 0),
                    stop=(ct == n_ct - 1),
                )
            nc.vector.tensor_copy(
                v_sb[:, st, :, 0:HEAD_DIM],
                ps[:, 0:DH].rearrange("p (h d) -> p h d", h=HEADS_PER_CORE),
            )

        # first query block for chunk0 heads; the rest are deferred into the
        # early attention pipeline slots
        emit_qk_group(wq_sb, qT_sb, 0, 0)
        proj_scope.__exit__(None, None, None)
        # ---- attention: software-pipelined head-pair chunk loop ----
        # One slot = one s-tile for a pair of heads (hp): the two K=64 score
        # matmuls land on disjoint PE row groups (base partitions 0 and 64)
        # and run concurrently; both scores share one [128,1024] PSUM tile so
        # a single ACTIVATE computes exp for both heads.
        heads = list(range(HEADS_PER_CORE))
        chunk_list = []
        for tb in range(n_tb):
            for hp in range(HEADS_PER_CORE // 2):
                for c in range(n_st):
                    chunk_list.append((tb, hp, c))
        n_total = len(chunk_list)

        deferred = {}

        def defer(slot, fn):
            deferred.setdefault(slot, []).append(fn)

        o_ps_map = {}
        denom_map = {}

        def emit_S_ACT(i):
            tb, hp, st = chunk_list[i]
            chunk_hd = hp
            if st == 0:
                o_psA = psum_o.tile([HEAD_DIM + 1, 512], F32, tag="o")
                o_psB = psum_o.tile([HEAD_DIM + 1, 512], F32, tag="o")
                o_ps_map[(tb, 2 * hp)] = o_psA
                o_ps_map[(tb, 2 * hp + 1)] = o_psB
            with nc.named_scope("S"):
                ps = psum_s.tile([128, 2 * 512], F32, tag="s")
                for half, dlo in ((0, 0), (1, 64)):
                    nc.tensor.matmul(
                        ps[:, half * 512 : (half + 1) * 512],
                        lhsT=kT_sb[dlo : dlo + 64, chunk_hd, st * 128 : (st + 1) * 128],
                        rhs=qT_sb[dlo : dlo + 64, chunk_hd, tb * 512 : (tb + 1) * 512],
                        start=True,
                        stop=True,
                    )
            with nc.named_scope("exp"):
                pt = pt_pool.tile([128, 2, 512], BF16)
                nc.scalar.activation(
                    pt.rearrange("p a b -> p (a b)"), ps, EXP, scale=scale
                )
            return ps, pt

        def emit_O(i, pt):
            tb, hp, st = chunk_list[i]
            with nc.named_scope("O"):
                for half in range(2):
                    h = 2 * hp + half
                    nc.tensor.matmul(
                        o_ps_map[(tb, h)],
                        lhsT=v_sb[:, st, h, :],
                        rhs=pt[:, half, :],
                        start=(st == 0),
                        stop=(st == n_st - 1),
                    )
            if st == n_st - 1:
                # pair (tb, hp) fully accumulated: drain PSUM immediately so
                # the banks recycle for the next pair (unnormalized O + denom)
                for half in range(2):
                    h = 2 * hp + half
                    o_ps = o_ps_map[(tb, h)]
                    o_unnorm = onorm_pool.tile([64, 512], BF16, tag="ou")
                    nc.vector.tensor_copy(o_unnorm, o_ps[0:64, :])
                    o_unnorm_map[(tb, h)] = o_unnorm
                    denom_f = small.tile([1, 512], F32, tag="denom_f")
                    nc.vector.tensor_copy(denom_f, o_ps[64:65, :])
                    denom_map[(tb, h)] = denom_f

        o_unnorm_map = {}

        def emit_norm(tb, h):
            ns = nc.named_scope("norm"); ns.__enter__()
            chunk_hd, dlo = h // 2, (h % 2) * 64
            recip_f = small.tile([1, 512], F32, tag="recip_f")
            nc.vector.reciprocal_approx_fast(recip_f, denom_map[(tb, h)])
            recip = small.tile([1, 512], BF16, tag="recip")
            nc.vector.tensor_copy(recip, recip_f)
            rep = psum_f.tile([128, 512], F32, tag="f")
            nc.tensor.matmul(
                rep[0:64, :], lhsT=ones_sb, rhs=recip, start=True, stop=True
            )
            rep_sb = small.tile([64, 512], BF16, tag="rep")
            nc.vector.tensor_copy(rep_sb, rep[0:64, :])
            nc.vector.tensor_mul(
                oT_sb[dlo : dlo + 64, chunk_hd, tb * 512 : (tb + 1) * 512],
                o_unnorm_map[(tb, h)],
                rep_sb,
            )
            ns.__exit__(None, None, None)

        def emit_final(tb):
            ns = nc.named_scope("final"); ns.__enter__()
            # y partial for query block tb: 4 t-tiles of 128
            y_tiled = y.rearrange("(tt p) c -> p tt c", p=128)
            for tt in range(tb * 4, tb * 4 + 4):
                for cb in range(n_cb):
                    ps = psum_f.tile([128, 512], F32, tag="f")
                    for chunk in range(2):
                        nc.tensor.matmul(
                            ps,
                            lhsT=oT_sb[:, chunk, tt * 128 : (tt + 1) * 128],
                            rhs=wo_sb[:, chunk, cb * 512 : (cb + 1) * 512],
                            start=(chunk == 0),
                            stop=(chunk == 1),
                        )
                    st_out = out_stage.tile([128, 512], F32)
                    nc.vector.tensor_copy(st_out, ps)
                    nc.sync.dma_start(
                        out=y_tiled[:, tt, cb * 512 : (cb + 1) * 512], in_=st_out
                    )
            ns.__exit__(None, None, None)

        # schedule: at slot i emit S+ACT(i) then O(i-1); normalization for tb
        # lands ~3 chunks into tb+1, final projection ~6 chunks in.
        # remaining projection groups dripped into early pipeline slots
        rest = [(wk_sb, kT_sb, 1, tb) for tb in range(n_tb)]
        rest += [(wq_sb, qT_sb, 1, 0)]
        for tb in range(1, n_tb):
            rest += [(wq_sb, qT_sb, 0, tb), (wq_sb, qT_sb, 1, tb)]
        for idx, (w_sb_, dst_, chunk_, tb_) in enumerate(rest):
            defer(
                1 + 2 * idx,
                lambda w=w_sb_, d=dst_, c=chunk_, t=tb_: emit_qk_group(w, d, c, t),
            )

        per_tb = 2 * n_st
        last = n_total + 1  # after the final pending O pops
        for tb in range(n_tb):
            for hp in range(2):
                pair_end = (tb * 2 + hp + 1) * n_st
                defer(min(pair_end + 3, last), lambda tb=tb, h=2 * hp: emit_norm(tb, h))
                defer(
                    min(pair_end + 4, last),
                    lambda tb=tb, h=2 * hp + 1: emit_norm(tb, h),
                )
            end_slot = (tb + 1) * per_tb
            defer(min(end_slot + 8, last), lambda tb=tb: emit_final(tb))

        from collections import deque

        pending = deque()
        DEPTH = 2
        for i in range(n_total + DEPTH):
            if i < n_total:
                ps, pt = emit_S_ACT(i)
                pending.append((i, pt))
            if len(pending) > DEPTH or i >= n_total:
                j, jpt = pending.popleft()
                emit_O(j, jpt)
            for fn in deferred.get(i, ()):
                fn()

    nc.compile()
    return nc


def make_core_inputs(x, wq, wk, wv, wo):
    """Shard + pre-layout the full inputs into 8 per-core input maps."""
    bf = ml_dtypes.bfloat16
    in_maps = []
    for core in range(N_CORES):
        b = core // 4
        g = core % 4
        lo, hi = g * DH, (g + 1) * DH
        in_maps.append(
            {
                "xT": np.ascontiguousarray(x[b].T).astype(bf),
                "wqT": np.ascontiguousarray(wq[lo:hi, :].T).astype(bf),
                "wkT": np.ascontiguousarray(wk[lo:hi, :].T).astype(bf),
                "wvT": np.ascontiguousarray(wv[lo:hi, :].T).astype(bf),
                "woT": np.ascontiguousarray(wo[:, lo:hi].T).astype(bf),
            }
        )
    return in_maps


_PROGRAM_CACHE = {}


def _get_program():
    if "nc" not in _PROGRAM_CACHE:
        nc = build_program()
        nc.m = get_hw_module(nc.m)
        _PROGRAM_CACHE["nc"] = nc
    return _PROGRAM_CACHE["nc"]


def run_sharded(in_maps, trace=False):
    nc = _get_program()
    return bass_utils.run_bass_kernel_spmd(
        nc, in_maps, core_ids=list(range(N_CORES)), trace=trace
    )


def kernel(x, wq, wk, wv, wo):
    x = np.asarray(x, dtype=np.float32)
    wq = np.asarray(wq, dtype=np.float32)
    wk = np.asarray(wk, dtype=np.float32)
    wv = np.asarray(wv, dtype=np.float32)
    wo = np.asarray(wo, dtype=np.float32)

    in_maps = make_core_inputs(x, wq, wk, wv, wo)
    res = run_sharded(in_maps)

    B, T, C = x.shape
    out = np.zeros((B, T, C), dtype=np.float32)
    for core in range(N_CORES):
        out[core // 4] += res.results[core]["y"]
    return out


if __name__ == "__main__":
    rng = np.random.default_rng(0)
    x = rng.standard_normal((2, 2048, 1024), dtype=np.float32)
    s = 1.0 / np.sqrt(N_EMBD)
    ws = [rng.standard_normal((1024, 1024), dtype=np.float32) * s for _ in range(4)]
    out = kernel(x, *ws)
    print("out", out.shape, out.dtype, float(np.abs(out).max()))


# revision 26
# speedup vs baseline: 1.1039x; 1.0003x over previous
"""Multi-head attention kernel for 8 TRN2 NeuronCores (Bass/Tile).

Problem: x[2,2048,1024], 16 heads x 64 dims, torch-style Linear weights.
Sharding: data parallel over batch (2) x tensor parallel over heads (16/4):
core c handles batch c//4, heads 4*(c%4) .. 4*(c%4)+3. Each core computes
its heads' attention output projected through its slice of wo, producing a
partial [2048, 1024] fp32 output; the host sums the 4 partials per batch
(the "all-reduce after wo").

Device dataflow per core (matmul operands bf16, fp32 accumulation):
  QT/KT = weight-slice projections in [d, t] layout (d on partitions)
  V     = projection in natural [s, d] layout, with a ones column appended
          per head so the P@V matmul also yields the softmax denominator
  S^T   = K^T.T @ Q^T per head ([s, t] layout, s on partitions)
  P^T   = exp(S^T / 8) via ScalarE (no max subtraction: logits are O(8))
  O^T   = V.T @ P^T accumulated over s in PSUM (row 64 = denominator)
  y     = (O^T / denom)^T @ wo-slice^T, partial over this core's heads

The attention stage is ScalarE(exp)-bound (~16.8M exp/core), so the s-loop
is software-pipelined in chunks of 2 s-tiles: the PE issues chunk c+1's
score matmuls before chunk c's PV matmuls so the in-order PE queue never
stalls on the exp dependency, and each exp is one [128,1024] PSUM-source
ACTIVATE (amortizes the per-instruction bubble). Final projection and the
softmax normalization for query-block tb are deferred into tb+1's pipeline.
"""

import sys

sys.path.insert(0, "/opt/trn_rl_repo")

from contextlib import ExitStack

import ml_dtypes
import numpy as np

import concourse.bass as bass
import concourse.tile as tile
from concourse import bacc, mybir
from concourse import bass_utils
from concourse.bass_interp import get_hw_module

BF16 = mybir.dt.bfloat16
F32 = mybir.dt.float32
EXP = mybir.ActivationFunctionType.Exp

N_EMBD = 1024
N_HEAD = 16
HEAD_DIM = 64

N_CORES = 8
HEADS_PER_CORE = 4
DH = HEADS_PER_CORE * HEAD_DIM  # 256
CH = 2  # s-tiles per exp chunk


def build_program(T=2048, C=N_EMBD, enable_asserts=False):
    nc = bacc.Bacc(
        "TRN2", target_bir_lowering=False, debug=False, enable_asserts=enable_asserts
    )

    xT = nc.dram_tensor("xT", [C, T], BF16, kind="ExternalInput").ap()
    wqT = nc.dram_tensor("wqT", [C, DH], BF16, kind="ExternalInput").ap()
    wkT = nc.dram_tensor("wkT", [C, DH], BF16, kind="ExternalInput").ap()
    wvT = nc.dram_tensor("wvT", [C, DH], BF16, kind="ExternalInput").ap()
    woT = nc.dram_tensor("woT", [DH, C], BF16, kind="ExternalInput").ap()
    y = nc.dram_tensor("y", [T, C], F32, kind="ExternalOutput").ap()

    n_ct = C // 128   # contraction tiles over embedding dim
    n_st = T // 128   # s tiles (key/value positions)
    n_tb = T // 512   # query blocks
    n_cb = C // 512   # output column blocks
    n_ck = n_st // CH  # exp chunks per (h, tb)

    scale = float(HEAD_DIM**-0.5)

    with tile.TileContext(nc) as tc, ExitStack() as ctx:
        statics = ctx.enter_context(tc.tile_pool(name="statics", bufs=1))
        pt_pool = ctx.enter_context(tc.tile_pool(name="pt", bufs=4))
        onorm_pool = ctx.enter_context(tc.tile_pool(name="onorm", bufs=6))
        small = ctx.enter_context(tc.tile_pool(name="small", bufs=6))
        out_stage = ctx.enter_context(tc.tile_pool(name="out_stage", bufs=4))

        psum_s = ctx.enter_context(tc.tile_pool(name="psum_s", bufs=2, space="PSUM"))
        psum_o = ctx.enter_context(tc.tile_pool(name="psum_o", bufs=2, space="PSUM"))
        psum_f = ctx.enter_context(tc.tile_pool(name="psum_f", bufs=2, space="PSUM"))

        # ---- static SBUF tensors ----
        xT_sb = statics.tile([128, n_ct, T], BF16)
        wq_sb = statics.tile([128, n_ct, DH], BF16)
        wk_sb = statics.tile([128, n_ct, DH], BF16)
        wv_sb = statics.tile([128, n_ct, DH], BF16)
        wo_sb = statics.tile([128, 2, C], BF16)
        qT_sb = statics.tile([128, 2, T], BF16)
        kT_sb = statics.tile([128, 2, T], BF16)
        v_sb = statics.tile([128, n_st, HEADS_PER_CORE, HEAD_DIM + 1], BF16)
        oT_sb = statics.tile([128, 2, T], BF16)
        ones_sb = statics.tile([1, 64], BF16)

        nc.sync.dma_start(out=wk_sb, in_=wkT.rearrange("(a p) d -> p a d", p=128))
        nc.sync.dma_start(out=wv_sb, in_=wvT.rearrange("(a p) d -> p a d", p=128))
        xT_chunked = xT.rearrange("(a p) t -> p a t", p=128)
        for ct in range(n_ct):
            half = T // 2
            nc.sync.dma_start(out=xT_sb[:, ct, 0:half], in_=xT_chunked[:, ct, 0:half])
            nc.sync.dma_start(out=xT_sb[:, ct, half:T], in_=xT_chunked[:, ct, half:T])
        nc.sync.dma_start(out=wq_sb, in_=wqT.rearrange("(a p) d -> p a d", p=128))
        nc.sync.dma_start(out=wo_sb, in_=woT.rearrange("(a p) c -> p a c", p=128))
        nc.vector.memset(ones_sb, 1.0)
        nc.vector.memset(v_sb[:, :, :, HEAD_DIM : HEAD_DIM + 1], 1.0)

        # ---- projections (overlapped with the xT DMA fill) ----
        proj_scope = nc.named_scope("proj")
        proj_scope.__enter__()

        def emit_qk_group(w_sb, dst, chunk, tb):
            ps = psum_f.tile([128, 512], F32, tag="f")
            for ct in range(n_ct):
                nc.tensor.matmul(
                    ps,
                    lhsT=w_sb[:, ct, chunk * 128 : (chunk + 1) * 128],
                    rhs=xT_sb[:, ct, tb * 512 : (tb + 1) * 512],
                    start=(ct == 0),
                    stop=(ct == n_ct - 1),
                )
            nc.vector.tensor_copy(dst[:, chunk, tb * 512 : (tb + 1) * 512], ps)

        def emit_v_pair(p):
            # V projection for s-tiles 2p, 2p+1 (natural layout + ones col);
            # two tiles: a PSUM accumulation group owns its whole bank
            psA = psum_f.tile([128, 512], F32, tag="f")
            psB = psum_f.tile([128, 512], F32, tag="f")
            vps = [psA, psB]
            for ct in range(n_ct):
                for k in range(2):
                    st = 2 * p + k
                    nc.tensor.matmul(
                        vps[k][:, 0:DH],
                        lhsT=xT_sb[:, ct, st * 128 : (st + 1) * 128],
                        rhs=wv_sb[:, ct, :],
                        start=(ct == 0),
                        stop=(ct == n_ct - 1),
                    )
            for k in range(2):
                st = 2 * p + k
                nc.vector.tensor_copy(
                    v_sb[:, st, :, 0:HEAD_DIM],
                    vps[k][:, 0:DH].rearrange("p (h d) -> p h d", h=HEADS_PER_CORE),
                )

        # K chunk0 (4 query-block groups) and the first V pair accumulate
        # ct-outer so the matmuls chase the per-chunk xT DMAs as they land
        kps0 = psum_s.tile([128, 1024], F32, tag="s")
        kps1 = psum_s.tile([128, 1024], F32, tag="s")
        kps = [kps0, kps1]
        vpsA = psum_f.tile([128, 512], F32, tag="f")
        vpsB = psum_f.tile([128, 512], F32, tag="f")
        vps0 = [vpsA, vpsB]
        for ct in range(n_ct):
            for tb in range(n_tb):
                nc.tensor.matmul(
                    kps[tb // 2][:, (tb % 2) * 512 : (tb % 2 + 1) * 512],
                    lhsT=wk_sb[:, ct, 0:128],
                    rhs=xT_sb[:, ct, tb * 512 : (tb + 1) * 512],
                    start=(ct == 0),
                    stop=(ct == n_ct - 1),
                )
            for k in range(2):
                nc.tensor.matmul(
                    vps0[k][:, 0:DH],
                    lhsT=xT_sb[:, ct, k * 128 : (k + 1) * 128],
                    rhs=wv_sb[:, ct, :],
                    start=(ct == 0),
                    stop=(ct == n_ct - 1),
                )
        for tb in range(n_tb):
            nc.vector.tensor_copy(
                kT_sb[:, 0, tb * 512 : (tb + 1) * 512],
                kps[tb // 2][:, (tb % 2) * 512 : (tb % 2 + 1) * 512],
            )
        for k in range(2):
            nc.vector.tensor_copy(
                v_sb[:, k, :, 0:HEAD_DIM],
                vps0[k][:, 0:DH].rearrange("p (h d) -> p h d", h=HEADS_PER_CORE),
            )

        # first query block for chunk0 heads; everything else (remaining V
        # pairs, K chunk1, other Q blocks) drips into the pipeline slots
        emit_qk_group(wq_sb, qT_sb, 0, 0)
        proj_scope.__exit__(None, None, None)
        # ---- attention: software-pipelined head-pair chunk loop ----
        # One slot = one s-tile for a pair of heads (hp): the two K=64 score
        # matmuls land on disjoint PE row groups (base partitions 0 and 64)
        # and run concurrently; both scores share one [128,1024] PSUM tile so
        # a single ACTIVATE computes exp for both heads.
        heads = list(range(HEADS_PER_CORE))
        chunk_list = []
        for hp in range(HEADS_PER_CORE // 2):
            for tb in range(n_tb):
                for c in range(n_st):
                    chunk_list.append((tb, hp, c))
        n_total = len(chunk_list)

        deferred = {}

        def defer(slot, fn):
            deferred.setdefault(slot, []).append(fn)

        o_ps_map = {}
        denom_map = {}

        def emit_S_ACT(i):
            tb, hp, st = chunk_list[i]
            chunk_hd = hp
            if st == 0:
                o_psA = psum_o.tile([HEAD_DIM + 1, 512], F32, tag="o")
                o_psB = psum_o.tile([HEAD_DIM + 1, 512], F32, tag="o")
                o_ps_map[(tb, 2 * hp)] = o_psA
                o_ps_map[(tb, 2 * hp + 1)] = o_psB
            with nc.named_scope("S"):
                ps = psum_s.tile([128, 2 * 512], F32, tag="s")
                for half, dlo in ((0, 0), (1, 64)):
                    nc.tensor.matmul(
                        ps[:, half * 512 : (half + 1) * 512],
                        lhsT=kT_sb[dlo : dlo + 64, chunk_hd, st * 128 : (st + 1) * 128],
                        rhs=qT_sb[dlo : dlo + 64, chunk_hd, tb * 512 : (tb + 1) * 512],
                        start=True,
                        stop=True,
                    )
            with nc.named_scope("exp"):
                pt = pt_pool.tile([128, 2, 512], BF16)
                nc.scalar.activation(
                    pt.rearrange("p a b -> p (a b)"), ps, EXP, scale=scale
                )
            return ps, pt

        def emit_O(i, pt):
            tb, hp, st = chunk_list[i]
            with nc.named_scope("O"):
                for half in range(2):
                    h = 2 * hp + half
                    nc.tensor.matmul(
                        o_ps_map[(tb, h)],
                        lhsT=v_sb[:, st, h, :],
                        rhs=pt[:, half, :],
                        start=(st == 0),
                        stop=(st == n_st - 1),
                    )
            if st == n_st - 1:
                # pair (tb, hp) fully accumulated: drain PSUM immediately so
                # the banks recycle for the next pair (unnormalized O + denom)
                for half in range(2):
                    h = 2 * hp + half
                    o_ps = o_ps_map[(tb, h)]
                    o_unnorm = onorm_pool.tile([64, 512], BF16, tag="ou")
                    nc.vector.tensor_copy(o_unnorm, o_ps[0:64, :])
                    o_unnorm_map[(tb, h)] = o_unnorm
                    denom_f = small.tile([1, 512], F32, tag="denom_f")
                    nc.vector.tensor_copy(denom_f, o_ps[64:65, :])
                    denom_map[(tb, h)] = denom_f

        o_unnorm_map = {}

        def emit_norm(tb, h):
            ns = nc.named_scope("norm"); ns.__enter__()
            chunk_hd, dlo = h // 2, (h % 2) * 64
            recip_f = small.tile([1, 512], F32, tag="recip_f")
            nc.vector.reciprocal_approx_fast(recip_f, denom_map[(tb, h)])
            recip = small.tile([1, 512], BF16, tag="recip")
            nc.vector.tensor_copy(recip, recip_f)
            rep = psum_f.tile([128, 512], F32, tag="f")
            nc.tensor.matmul(
                rep[0:64, :], lhsT=ones_sb, rhs=recip, start=True, stop=True
            )
            rep_sb = small.tile([64, 512], BF16, tag="rep")
            nc.vector.tensor_copy(rep_sb, rep[0:64, :])
            nc.vector.tensor_mul(
                oT_sb[dlo : dlo + 64, chunk_hd, tb * 512 : (tb + 1) * 512],
                o_unnorm_map[(tb, h)],
                rep_sb,
            )
            ns.__exit__(None, None, None)

        def emit_final(tb):
            ns = nc.named_scope("final"); ns.__enter__()
            # y partial for query block tb: 4 t-tiles of 128
            y_tiled = y.rearrange("(tt p) c -> p tt c", p=128)
            for tt in range(tb * 4, tb * 4 + 4):
                for cb in range(n_cb):
                    ps = psum_f.tile([128, 512], F32, tag="f")
                    for chunk in range(2):
                        nc.tensor.matmul(
                            ps,
                            lhsT=oT_sb[:, chunk, tt * 128 : (tt + 1) * 128],
                            rhs=wo_sb[:, chunk, cb * 512 : (cb + 1) * 512],
                            start=(chunk == 0),
                            stop=(chunk == 1),
                        )
                    st_out = out_stage.tile([128, 512], F32)
                    nc.vector.tensor_copy(st_out, ps)
                    nc.sync.dma_start(
                        out=y_tiled[:, tt, cb * 512 : (cb + 1) * 512], in_=st_out
                    )
            ns.__exit__(None, None, None)

        last = n_total + 1  # after the final pending O pops
        hp_span = n_tb * n_st  # slots per head-pair sweep

        # remaining V pairs: pair p (s-tiles 2p,2p+1) is consumed by the O
        # matmuls of slot 2p (which pop at 2p+2)
        for p in range(1, n_st // 2):
            defer(2 * p - 1, lambda p=p: emit_v_pair(p))
        # remaining Q/K projection groups, scheduled ahead of first use:
        # Q block (hp, tb) is first read at slot hp*hp_span + tb*n_st; K
        # chunk1 s-block g is first read at slot hp_span + 4*g.
        qk_rest = []
        for tb in range(1, n_tb):
            qk_rest.append((wq_sb, qT_sb, 0, tb, tb * n_st - 8))
        for g in range(n_tb):
            qk_rest.append((wk_sb, kT_sb, 1, g, hp_span + 4 * g - 24))
        for tb in range(n_tb):
            qk_rest.append((wq_sb, qT_sb, 1, tb, hp_span + tb * n_st - 12))
        for w_sb_, dst_, chunk_, tb_, slot in qk_rest:
            defer(
                max(1, min(slot, last)),
                lambda w=w_sb_, d=dst_, c=chunk_, t=tb_: emit_qk_group(w, d, c, t),
            )

        for hp in range(2):
            for tb in range(n_tb):
                pair_end = (hp * n_tb + tb + 1) * n_st
                defer(min(pair_end + 3, last), lambda tb=tb, h=2 * hp: emit_norm(tb, h))
                defer(
                    min(pair_end + 4, last),
                    lambda tb=tb, h=2 * hp + 1: emit_norm(tb, h),
                )
                if hp == 1:
                    defer(min(pair_end + 8, last), lambda tb=tb: emit_final(tb))

        from collections import deque

        pending = deque()
        DEPTH = 2
        for i in range(n_total + DEPTH):
            if i < n_total:
                ps, pt = emit_S_ACT(i)
                pending.append((i, pt))
            if len(pending) > DEPTH or i >= n_total:
                j, jpt = pending.popleft()
                emit_O(j, jpt)
            for fn in deferred.get(i, ()):
                fn()

    nc.compile()
    return nc


def make_core_inputs(x, wq, wk, wv, wo):
    """Shard + pre-layout the full inputs into 8 per-core input maps."""
    bf = ml_dtypes.bfloat16
    in_maps = []
    for core in range(N_CORES):
        b = core // 4
        g = core % 4
        lo, hi = g * DH, (g + 1) * DH
        in_maps.append(
            {
                "xT": np.ascontiguousarray(x[b].T).astype(bf),
                "wqT": np.ascontiguousarray(wq[lo:hi, :].T).astype(bf),
                "wkT": np.ascontiguousarray(wk[lo:hi, :].T).astype(bf),
                "wvT": np.ascontiguousarray(wv[lo:hi, :].T).astype(bf),
                "woT": np.ascontiguousarray(wo[:, lo:hi].T).astype(bf),
            }
        )
    return in_maps


_PROGRAM_CACHE = {}


def _get_program():
    if "nc" not in _PROGRAM_CACHE:
        nc = build_program()
        nc.m = get_hw_module(nc.m)
        _PROGRAM_CACHE["nc"] = nc
    return _PROGRAM_CACHE["nc"]


def run_sharded(in_maps, trace=False):
    nc = _get_program()
    return bass_utils.run_bass_kernel_spmd(
        nc, in_maps, core_ids=list(range(N_CORES)), trace=trace
    )


def kernel(x, wq, wk, wv, wo):
    x = np.asarray(x, dtype=np.float32)
    wq = np.asarray(wq, dtype=np.float32)
    wk = np.asarray(wk, dtype=np.float32)
    wv = np.asarray(wv, dtype=np.float32)
    wo = np.asarray(wo, dtype=np.float32)

    in_maps = make_core_inputs(x, wq, wk, wv, wo)
    res = run_sharded(in_maps)

    B, T, C = x.shape
    out = np.zeros((B, T, C), dtype=np.float32)
    for core in range(N_CORES):
        out[core // 4] += res.results[core]["y"]
    return out


if __name__ == "__main__":
    rng = np.random.default_rng(0)
    x = rng.standard_normal((2, 2048, 1024), dtype=np.float32)
    s = 1.0 / np.sqrt(N_EMBD)
    ws = [rng.standard_normal((1024, 1024), dtype=np.float32) * s for _ in range(4)]
    out = kernel(x, *ws)
    print("out", out.shape, out.dtype, float(np.abs(out).max()))


# revision 28
# speedup vs baseline: 1.1247x; 1.0189x over previous
"""Multi-head attention kernel for 8 TRN2 NeuronCores (Bass/Tile).

Problem: x[2,2048,1024], 16 heads x 64 dims, torch-style Linear weights.
Sharding: data parallel over batch (2) x tensor parallel over heads (16/4):
core c handles batch c//4, heads 4*(c%4) .. 4*(c%4)+3. Each core computes
its heads' attention output projected through its slice of wo, producing a
partial [2048, 1024] fp32 output; the host sums the 4 partials per batch
(the "all-reduce after wo").

Device dataflow per core (matmul operands bf16, fp32 accumulation):
  QT/KT = weight-slice projections in [d, t] layout (d on partitions)
  V     = projection in natural [s, d] layout, with a ones column appended
          per head so the P@V matmul also yields the softmax denominator
  S^T   = K^T.T @ Q^T per head ([s, t] layout, s on partitions)
  P^T   = exp(S^T / 8) via ScalarE (no max subtraction: logits are O(8))
  O^T   = V.T @ P^T accumulated over s in PSUM (row 64 = denominator)
  y     = (O^T / denom)^T @ wo-slice^T, partial over this core's heads

The attention stage is ScalarE(exp)-bound (~16.8M exp/core), so the s-loop
is software-pipelined in chunks of 2 s-tiles: the PE issues chunk c+1's
score matmuls before chunk c's PV matmuls so the in-order PE queue never
stalls on the exp dependency, and each exp is one [128,1024] PSUM-source
ACTIVATE (amortizes the per-instruction bubble). Final projection and the
softmax normalization for query-block tb are deferred into tb+1's pipeline.
"""

import sys

sys.path.insert(0, "/opt/trn_rl_repo")

from contextlib import ExitStack

import ml_dtypes
import numpy as np

import concourse.bass as bass
import concourse.tile as tile
from concourse import bacc, mybir
from concourse import bass_utils
from concourse.bass_interp import get_hw_module

BF16 = mybir.dt.bfloat16
F32 = mybir.dt.float32
EXP = mybir.ActivationFunctionType.Exp

N_EMBD = 1024
N_HEAD = 16
HEAD_DIM = 64

N_CORES = 8
HEADS_PER_CORE = 4
DH = HEADS_PER_CORE * HEAD_DIM  # 256
CH = 2  # s-tiles per exp chunk


def build_program(T=2048, C=N_EMBD, enable_asserts=False):
    nc = bacc.Bacc(
        "TRN2", target_bir_lowering=False, debug=False, enable_asserts=enable_asserts
    )

    xT = nc.dram_tensor("xT", [C, T], BF16, kind="ExternalInput").ap()
    wqT = nc.dram_tensor("wqT", [C, DH], BF16, kind="ExternalInput").ap()
    wkT = nc.dram_tensor("wkT", [C, DH], BF16, kind="ExternalInput").ap()
    wvT = nc.dram_tensor("wvT", [C, DH], BF16, kind="ExternalInput").ap()
    woT = nc.dram_tensor("woT", [DH, C], BF16, kind="ExternalInput").ap()
    y = nc.dram_tensor("y", [T, C], F32, kind="ExternalOutput").ap()

    n_ct = C // 128   # contraction tiles over embedding dim
    n_st = T // 128   # s tiles (key/value positions)
    n_tb = T // 512   # query blocks
    n_cb = C // 512   # output column blocks
    n_ck = n_st // CH  # exp chunks per (h, tb)

    scale = float(HEAD_DIM**-0.5)

    with tile.TileContext(nc) as tc, ExitStack() as ctx:
        statics = ctx.enter_context(tc.tile_pool(name="statics", bufs=1))
        pt_pool = ctx.enter_context(tc.tile_pool(name="pt", bufs=4))
        onorm_pool = ctx.enter_context(tc.tile_pool(name="onorm", bufs=6))
        small = ctx.enter_context(tc.tile_pool(name="small", bufs=6))
        out_stage = ctx.enter_context(tc.tile_pool(name="out_stage", bufs=4))

        psum_s = ctx.enter_context(tc.tile_pool(name="psum_s", bufs=2, space="PSUM"))
        psum_o = ctx.enter_context(tc.tile_pool(name="psum_o", bufs=2, space="PSUM"))
        psum_f = ctx.enter_context(tc.tile_pool(name="psum_f", bufs=2, space="PSUM"))

        # ---- static SBUF tensors ----
        xT_sb = statics.tile([128, n_ct, T], BF16)
        wq_sb = statics.tile([128, n_ct, DH], BF16)
        wk_sb = statics.tile([128, n_ct, DH], BF16)
        wv_sb = statics.tile([128, n_ct, DH], BF16)
        wo_sb = statics.tile([128, 2, C], BF16)
        qT_sb = statics.tile([128, 2, T], BF16)
        kT_sb = statics.tile([128, 2, T], BF16)
        v_sb = statics.tile([128, n_st, HEADS_PER_CORE, HEAD_DIM + 1], BF16)
        oT_sb = statics.tile([128, 2, T], BF16)
        ones_sb = statics.tile([1, 64], BF16)

        nc.sync.dma_start(out=wk_sb, in_=wkT.rearrange("(a p) d -> p a d", p=128))
        nc.sync.dma_start(out=wv_sb, in_=wvT.rearrange("(a p) d -> p a d", p=128))
        xT_chunked = xT.rearrange("(a p) t -> p a t", p=128)
        for ct in range(n_ct):
            half = T // 2
            nc.sync.dma_start(out=xT_sb[:, ct, 0:half], in_=xT_chunked[:, ct, 0:half])
            nc.sync.dma_start(out=xT_sb[:, ct, half:T], in_=xT_chunked[:, ct, half:T])
        nc.sync.dma_start(out=wq_sb, in_=wqT.rearrange("(a p) d -> p a d", p=128))
        nc.sync.dma_start(out=wo_sb, in_=woT.rearrange("(a p) c -> p a c", p=128))
        nc.vector.memset(ones_sb, 1.0)
        nc.vector.memset(v_sb[:, :, :, HEAD_DIM : HEAD_DIM + 1], 1.0)

        # ---- projections (overlapped with the xT DMA fill) ----
        proj_scope = nc.named_scope("proj")
        proj_scope.__enter__()

        def emit_qk_group(w_sb, dst, chunk, tb):
            ps = psum_f.tile([128, 512], F32, tag="f")
            for ct in range(n_ct):
                nc.tensor.matmul(
                    ps,
                    lhsT=w_sb[:, ct, chunk * 128 : (chunk + 1) * 128],
                    rhs=xT_sb[:, ct, tb * 512 : (tb + 1) * 512],
                    start=(ct == 0),
                    stop=(ct == n_ct - 1),
                )
            nc.vector.tensor_copy(dst[:, chunk, tb * 512 : (tb + 1) * 512], ps)

        def emit_v_pair(p):
            # V projection for s-tiles 2p, 2p+1 (natural layout + ones col);
            # two tiles: a PSUM accumulation group owns its whole bank
            psA = psum_f.tile([128, 512], F32, tag="f")
            psB = psum_f.tile([128, 512], F32, tag="f")
            vps = [psA, psB]
            for ct in range(n_ct):
                for k in range(2):
                    st = 2 * p + k
                    nc.tensor.matmul(
                        vps[k][:, 0:DH],
                        lhsT=xT_sb[:, ct, st * 128 : (st + 1) * 128],
                        rhs=wv_sb[:, ct, :],
                        start=(ct == 0),
                        stop=(ct == n_ct - 1),
                    )
            for k in range(2):
                st = 2 * p + k
                nc.vector.tensor_copy(
                    v_sb[:, st, :, 0:HEAD_DIM],
                    vps[k][:, 0:DH].rearrange("p (h d) -> p h d", h=HEADS_PER_CORE),
                )

        # K chunk0 (4 query-block groups) and the first V pair accumulate
        # ct-outer so the matmuls chase the per-chunk xT DMAs as they land
        kps0 = psum_s.tile([128, 1024], F32, tag="s")
        kps1 = psum_s.tile([128, 1024], F32, tag="s")
        kps = [kps0, kps1]
        vpsA = psum_f.tile([128, 512], F32, tag="f")
        vpsB = psum_f.tile([128, 512], F32, tag="f")
        vps0 = [vpsA, vpsB]
        for ct in range(n_ct):
            for tb in range(n_tb):
                nc.tensor.matmul(
                    kps[tb // 2][:, (tb % 2) * 512 : (tb % 2 + 1) * 512],
                    lhsT=wk_sb[:, ct, 0:128],
                    rhs=xT_sb[:, ct, tb * 512 : (tb + 1) * 512],
                    start=(ct == 0),
                    stop=(ct == n_ct - 1),
                )
            for k in range(2):
                nc.tensor.matmul(
                    vps0[k][:, 0:DH],
                    lhsT=xT_sb[:, ct, k * 128 : (k + 1) * 128],
                    rhs=wv_sb[:, ct, :],
                    start=(ct == 0),
                    stop=(ct == n_ct - 1),
                )
        for tb in range(n_tb):
            nc.vector.tensor_copy(
                kT_sb[:, 0, tb * 512 : (tb + 1) * 512],
                kps[tb // 2][:, (tb % 2) * 512 : (tb % 2 + 1) * 512],
            )
        for k in range(2):
            nc.vector.tensor_copy(
                v_sb[:, k, :, 0:HEAD_DIM],
                vps0[k][:, 0:DH].rearrange("p (h d) -> p h d", h=HEADS_PER_CORE),
            )

        # second V pair right behind the chase loop, then the first query
        # block for chunk0 heads; everything else (remaining V pairs, K
        # chunk1, other Q blocks) drips into the pipeline slots
        emit_v_pair(1)
        emit_qk_group(wq_sb, qT_sb, 0, 0)
        proj_scope.__exit__(None, None, None)
        # ---- attention: software-pipelined head-pair chunk loop ----
        # One slot = one s-tile for a pair of heads (hp): the two K=64 score
        # matmuls land on disjoint PE row groups (base partitions 0 and 64)
        # and run concurrently; both scores share one [128,1024] PSUM tile so
        # a single ACTIVATE computes exp for both heads.
        heads = list(range(HEADS_PER_CORE))
        chunk_list = []
        for hp in range(HEADS_PER_CORE // 2):
            for tb in range(n_tb):
                for c in range(n_st):
                    chunk_list.append((tb, hp, c))
        n_total = len(chunk_list)

        deferred = {}

        def defer(slot, fn):
            deferred.setdefault(slot, []).append(fn)

        o_ps_map = {}
        denom_map = {}

        def emit_S_ACT(i):
            tb, hp, st = chunk_list[i]
            chunk_hd = hp
            if st == 0:
                o_psA = psum_o.tile([HEAD_DIM + 1, 512], F32, tag="o")
                o_psB = psum_o.tile([HEAD_DIM + 1, 512], F32, tag="o")
                o_ps_map[(tb, 2 * hp)] = o_psA
                o_ps_map[(tb, 2 * hp + 1)] = o_psB
            with nc.named_scope("S"):
                ps = psum_s.tile([128, 2 * 512], F32, tag="s")
                for half, dlo in ((0, 0), (1, 64)):
                    nc.tensor.matmul(
                        ps[:, half * 512 : (half + 1) * 512],
                        lhsT=kT_sb[dlo : dlo + 64, chunk_hd, st * 128 : (st + 1) * 128],
                        rhs=qT_sb[dlo : dlo + 64, chunk_hd, tb * 512 : (tb + 1) * 512],
                        start=True,
                        stop=True,
                    )
            with nc.named_scope("exp"):
                pt = pt_pool.tile([128, 2, 512], BF16)
                nc.scalar.activation(
                    pt.rearrange("p a b -> p (a b)"), ps, EXP, scale=scale
                )
            return ps, pt

        def emit_O(i, pt):
            tb, hp, st = chunk_list[i]
            with nc.named_scope("O"):
                for half in range(2):
                    h = 2 * hp + half
                    nc.tensor.matmul(
                        o_ps_map[(tb, h)],
                        lhsT=v_sb[:, st, h, :],
                        rhs=pt[:, half, :],
                        start=(st == 0),
                        stop=(st == n_st - 1),
                    )
            if st == n_st - 1:
                # pair (tb, hp) fully accumulated: drain PSUM immediately so
                # the banks recycle for the next pair (unnormalized O + denom)
                for half in range(2):
                    h = 2 * hp + half
                    o_ps = o_ps_map[(tb, h)]
                    o_unnorm = onorm_pool.tile([64, 512], BF16, tag="ou")
                    nc.vector.tensor_copy(o_unnorm, o_ps[0:64, :])
                    o_unnorm_map[(tb, h)] = o_unnorm
                    denom_f = small.tile([1, 512], F32, tag="denom_f")
                    nc.vector.tensor_copy(denom_f, o_ps[64:65, :])
                    denom_map[(tb, h)] = denom_f

        o_unnorm_map = {}

        def emit_norm(tb, h):
            ns = nc.named_scope("norm"); ns.__enter__()
            chunk_hd, dlo = h // 2, (h % 2) * 64
            recip_f = small.tile([1, 512], F32, tag="recip_f")
            nc.vector.reciprocal_approx_fast(recip_f, denom_map[(tb, h)])
            recip = small.tile([1, 512], BF16, tag="recip")
            nc.vector.tensor_copy(recip, recip_f)
            rep = psum_f.tile([128, 512], F32, tag="f")
            nc.tensor.matmul(
                rep[0:64, :], lhsT=ones_sb, rhs=recip, start=True, stop=True
            )
            rep_sb = small.tile([64, 512], BF16, tag="rep")
            nc.vector.tensor_copy(rep_sb, rep[0:64, :])
            nc.vector.tensor_mul(
                oT_sb[dlo : dlo + 64, chunk_hd, tb * 512 : (tb + 1) * 512],
                o_unnorm_map[(tb, h)],
                rep_sb,
            )
            ns.__exit__(None, None, None)

        def emit_final(tb):
            ns = nc.named_scope("final"); ns.__enter__()
            # y partial for query block tb: 4 t-tiles of 128
            y_tiled = y.rearrange("(tt p) c -> p tt c", p=128)
            for tt in range(tb * 4, tb * 4 + 4):
                for cb in range(n_cb):
                    ps = psum_f.tile([128, 512], F32, tag="f")
                    for chunk in range(2):
                        nc.tensor.matmul(
                            ps,
                            lhsT=oT_sb[:, chunk, tt * 128 : (tt + 1) * 128],
                            rhs=wo_sb[:, chunk, cb * 512 : (cb + 1) * 512],
                            start=(chunk == 0),
                            stop=(chunk == 1),
                        )
                    st_out = out_stage.tile([128, 512], F32)
                    nc.vector.tensor_copy(st_out, ps)
                    nc.sync.dma_start(
                        out=y_tiled[:, tt, cb * 512 : (cb + 1) * 512], in_=st_out
                    )
            ns.__exit__(None, None, None)

        last = n_total + 1  # after the final pending O pops
        hp_span = n_tb * n_st  # slots per head-pair sweep

        # deferred work is split into ~2-matmul micro-closures, one per slot,
        # so the in-order PE queue never gets a burst ahead of an S matmul
        def defer_seq(start, closures):
            if start < 1:
                for fn in closures:
                    fn()  # small-T fallback: run in the front phase
                return
            for k, fn in enumerate(closures):
                defer(min(start + k, last), fn)

        def qk_group_closures(w_sb, dst, chunk, tb):
            cell = {}

            def mk_mm(c0):
                def f():
                    if "ps" not in cell:
                        ps_qk = psum_f.tile([128, 512], F32, tag="f")
                        cell["ps"] = ps_qk
                    ps_qk = cell["ps"]
                    for ct in range(c0, c0 + 2):
                        nc.tensor.matmul(
                            ps_qk,
                            lhsT=w_sb[:, ct, chunk * 128 : (chunk + 1) * 128],
                            rhs=xT_sb[:, ct, tb * 512 : (tb + 1) * 512],
                            start=(ct == 0),
                            stop=(ct == n_ct - 1),
                        )
                return f

            def cp():
                nc.vector.tensor_copy(
                    dst[:, chunk, tb * 512 : (tb + 1) * 512], cell["ps"]
                )

            return [mk_mm(0), mk_mm(2), mk_mm(4), mk_mm(6), cp]

        def v_pair_closures(p):
            cell = {}

            def mk(k, c0):
                def f():
                    key = "t%d" % k
                    if key not in cell:
                        ps_v = psum_f.tile([128, 512], F32, tag="f")
                        cell[key] = ps_v
                    ps_v = cell[key]
                    st = 2 * p + k
                    for ct in range(c0, c0 + 4):
                        nc.tensor.matmul(
                            ps_v[:, 0:DH],
                            lhsT=xT_sb[:, ct, st * 128 : (st + 1) * 128],
                            rhs=wv_sb[:, ct, :],
                            start=(ct == 0),
                            stop=(ct == n_ct - 1),
                        )
                return f

            def cp():
                for k in range(2):
                    st = 2 * p + k
                    nc.vector.tensor_copy(
                        v_sb[:, st, :, 0:HEAD_DIM],
                        cell["t%d" % k][:, 0:DH].rearrange(
                            "p (h d) -> p h d", h=HEADS_PER_CORE
                        ),
                    )

            return [mk(0, 0), mk(0, 4), mk(1, 0), mk(1, 4), cp]

        def final_closures(tb):
            y_tiled = y.rearrange("(tt p) c -> p tt c", p=128)
            out = []
            for tt in range(tb * 4, tb * 4 + 4):
                for cb in range(n_cb):
                    def f(tt=tt, cb=cb):
                        ns = nc.named_scope("final")
                        ns.__enter__()
                        ps_fin = psum_f.tile([128, 512], F32, tag="f")
                        for chunk in range(2):
                            nc.tensor.matmul(
                                ps_fin,
                                lhsT=oT_sb[:, chunk, tt * 128 : (tt + 1) * 128],
                                rhs=wo_sb[:, chunk, cb * 512 : (cb + 1) * 512],
                                start=(chunk == 0),
                                stop=(chunk == 1),
                            )
                        st_out = out_stage.tile([128, 512], F32)
                        nc.vector.tensor_copy(st_out, ps_fin)
                        nc.sync.dma_start(
                            out=y_tiled[:, tt, cb * 512 : (cb + 1) * 512], in_=st_out
                        )
                        ns.__exit__(None, None, None)

                    out.append(f)
            return out

        # V pairs: copy must be emitted before the O matmuls of slot 2p
        # (which pop at slot 2p+2)
        for p in range(2, n_st // 2):
            defer_seq(2 * p - 4, v_pair_closures(p))
        # Q chunk0 blocks for later query sweeps
        for tb in range(1, n_tb):
            defer_seq(tb * n_st - 7, qk_group_closures(wq_sb, qT_sb, 0, tb))
        # K chunk1 s-blocks (first read at hp_span + 4g)
        for g in range(n_tb):
            defer_seq(hp_span - 24 + 5 * g, qk_group_closures(wk_sb, kT_sb, 1, g))
        # Q chunk1 blocks (first read at hp_span + tb*n_st)
        q1_starts = [-5, 6, 15, 30]
        for tb in range(n_tb):
            defer_seq(
                hp_span + q1_starts[tb % 4],
                qk_group_closures(wq_sb, qT_sb, 1, tb),
            )

        for hp in range(2):
            for tb in range(n_tb):
                pair_end = (hp * n_tb + tb + 1) * n_st
                defer(min(pair_end + 2, last), lambda tb=tb, h=2 * hp: emit_norm(tb, h))
                defer(
                    min(pair_end + 3, last),
                    lambda tb=tb, h=2 * hp + 1: emit_norm(tb, h),
                )
                if hp == 1:
                    defer_seq(min(pair_end + 5, last), final_closures(tb))

        from collections import deque

        pending = deque()
        DEPTH = 2
        for i in range(n_total + DEPTH):
            if i < n_total:
                ps, pt = emit_S_ACT(i)
                pending.append((i, pt))
            if len(pending) > DEPTH or i >= n_total:
                j, jpt = pending.popleft()
                emit_O(j, jpt)
            for fn in deferred.get(i, ()):
                fn()

    nc.compile()
    return nc


def make_core_inputs(x, wq, wk, wv, wo):
    """Shard + pre-layout the full inputs into 8 per-core input maps."""
    bf = ml_dtypes.bfloat16
    in_maps = []
    for core in range(N_CORES):
        b = core // 4
        g = core % 4
        lo, hi = g * DH, (g + 1) * DH
        in_maps.append(
            {
                "xT": np.ascontiguousarray(x[b].T).astype(bf),
                "wqT": np.ascontiguousarray(wq[lo:hi, :].T).astype(bf),
                "wkT": np.ascontiguousarray(wk[lo:hi, :].T).astype(bf),
                "wvT": np.ascontiguousarray(wv[lo:hi, :].T).astype(bf),
                "woT": np.ascontiguousarray(wo[:, lo:hi].T).astype(bf),
            }
        )
    return in_maps


_PROGRAM_CACHE = {}


def _get_program():
    if "nc" not in _PROGRAM_CACHE:
        nc = build_program()
        nc.m = get_hw_module(nc.m)
        _PROGRAM_CACHE["nc"] = nc
    return _PROGRAM_CACHE["nc"]


def run_sharded(in_maps, trace=False):
    nc = _get_program()
    return bass_utils.run_bass_kernel_spmd(
        nc, in_maps, core_ids=list(range(N_CORES)), trace=trace
    )


def kernel(x, wq, wk, wv, wo):
    x = np.asarray(x, dtype=np.float32)
    wq = np.asarray(wq, dtype=np.float32)
    wk = np.asarray(wk, dtype=np.float32)
    wv = np.asarray(wv, dtype=np.float32)
    wo = np.asarray(wo, dtype=np.float32)

    in_maps = make_core_inputs(x, wq, wk, wv, wo)
    res = run_sharded(in_maps)

    B, T, C = x.shape
    out = np.zeros((B, T, C), dtype=np.float32)
    for core in range(N_CORES):
        out[core // 4] += res.results[core]["y"]
    return out


if __name__ == "__main__":
    rng = np.random.default_rng(0)
    x = rng.standard_normal((2, 2048, 1024), dtype=np.float32)
    s = 1.0 / np.sqrt(N_EMBD)
    ws = [rng.standard_normal((1024, 1024), dtype=np.float32) * s for _ in range(4)]
    out = kernel(x, *ws)
    print("out", out.shape, out.dtype, float(np.abs(out).max()))
